# revision 22
# baseline (speedup 1.0000x reference)
"""Trainium2 Bass kernel for Controller.predict_pairwise_prob (cumm='sum').

Math (per batch b, with T=512 timesteps, C=32 channels):
    a   = log(coref + overwrite)                       [T, C]
    bb  = log(coref)                                   [T, C]
    cum = cumsum_t log((1-overwrite)*(1-EPS) + EPS)    [T, C]
    out[t1, t2] = logsumexp_c(a[t1] + bb[t2] + cum[t2] - cum[t1]) * (t2 > t1)

Key identity: with u = a - cum, v = bb + cum and any per-t shifts s1, s2
that track the channel level (we use channel 0: s1 = u[0-th chan], which
stays within ~+-25 of every other channel, so exp stays inside fp32 range
while the +-170 growth of cum cancels):

    out[t1, t2] = log( sum_c exp(u[t1,c]-s1[t1]) * exp(v[t2,c]-s2[t2]) )
                  + s1[t1] + s2[t2]

i.e. a [T,C] x [C,T] matmul in exp space -> log -> rank-1 corrections.

Layout: everything lives as [channel(32 partitions), t(512 free)] so the
cumsum is a single free-dim scan and the pairwise product is a K=32 f32r
matmul with no operand transposes at matmul time. Inputs arrive packed
[t, (cor|ow)] and are transposed on the PE in 4 [128,96] chunks that also
carry cor+ow (for log(cor+ow)).

Sharding: data-parallel over batch, one batch element per NeuronCore.
"""

import numpy as np

import concourse.bacc as bacc
import concourse.tile as tile
from concourse import mybir
from concourse.bass_utils import run_bass_kernel_spmd

EPS = 1e-8
P = 128          # partitions / t-block size
T = 512          # timesteps
C = 32           # channels
NB = T // P      # 4 t-blocks
FP = mybir.dt.float32
FR = mybir.dt.float32r
ALU = mybir.AluOpType
AF = mybir.ActivationFunctionType

WRITE_ZEROS = False  # run_bass_kernel_spmd pre-zeroes ExternalOutputs

_CACHE = {}


def _build():
    import concourse.bacc as _bacc_mod
    import concourse.hw_specs as _hw

    _orig_tables = _hw.get_activation_tables
    _only = "natural_log_exp_and_others"

    def _patched(arch):
        tabs = _orig_tables(arch)
        return {k: (v if k == _only else set()) for k, v in tabs.items()}

    _bacc_mod.get_activation_tables = _patched
    nc = bacc.Bacc(
        "TRN2",
        target_bir_lowering=False,
        debug=False,
        enable_asserts=False,
        num_devices=8,
    )

    corow = nc.dram_tensor("corow", [T, 2 * C], FP, kind="ExternalInput").ap()
    ident = nc.dram_tensor("ident", [P, P], FP, kind="ExternalInput").ap()
    maskt = nc.dram_tensor("maskt", [P, P], FP, kind="ExternalInput").ap()
    oness = nc.dram_tensor("oness", [C, T], FP, kind="ExternalInput").ap()
    zeros = nc.dram_tensor("zeros", [P, T - P], FP, kind="ExternalInput").ap()
    m1s = nc.dram_tensor("m1s", [1, T], FR, kind="Internal").ap()
    out = nc.dram_tensor("out", [T, T], FP, kind="ExternalOutput").ap()

    with tile.TileContext(nc) as tc:
        _body(tc, out, corow, ident, maskt, oness, zeros, m1s)

    nc.compile()
    return nc


def _body(tc, out, corow, ident, maskt, oness, zeros, m1s):
    nc = tc.nc
    S = 3 * C  # per-block stripe in the packed tile: cor | ow | cor+ow
    with (
        tc.tile_pool(name="main", bufs=1) as pool,
        tc.tile_pool(name="pp", bufs=4) as pp,
        tc.tile_pool(name="ps", bufs=1, space="PSUM") as psum,
        tc.tile_pool(name="ps_s", bufs=4, space="PSUM") as psum_s,
    ):
        # ---- load packed [t, cor|ow] first (critical path) ----
        pk_t = pool.tile([P, NB * S], FP, tag="pk")
        pk3 = pk_t[:].rearrange("p (n s c) -> p n s c", n=NB, c=C)
        nc.gpsimd.dma_start(
            pk_t[:].rearrange("p (n x) -> p n x", n=NB)[:, :, : 2 * C],
            corow.rearrange("(n p) x -> p n x", p=P),
        )

        # ---- constants on the second HWDGE queue (scalar) ----
        ident_t = pool.tile([P, P], FP, tag="ident")
        nc.scalar.dma_start(ident_t[:], ident)
        mask_t = pool.tile([P, P], FP, tag="mask")
        nc.scalar.dma_start(mask_t[:], maskt)
        ones_t = pool.tile([C, T], FP, tag="oness")
        nc.scalar.dma_start(ones_t[:], oness)
        if WRITE_ZEROS:
            zero_t = pool.tile([P, T - P], FP, tag="zeros")
            nc.scalar.dma_start(zero_t[:], zeros)

        # cor+ow into slot 2, then w = ln(1-(1-EPS)*ow) in-place over slot 1
        # (cheap here: 128 lanes vs 32 after the transpose)
        nc.vector.tensor_add(pk3[:, :, 2, :], pk3[:, :, 0, :], pk3[:, :, 1, :])
        nc.scalar.activation(
            pk3[:, :, 1, :], pk3[:, :, 1, :], AF.Ln, bias=1.0, scale=-(1.0 - EPS)
        )

        # ---- transpose all three stripes per t-block: [128, 96] -> [96, 128] ----
        pk_ps = psum.tile([S, T], FP, tag="pkT")
        for n in range(NB):
            nc.tensor.transpose(
                pk_ps[:, P * n : P * (n + 1)],
                pk_t[:, S * n : S * (n + 1)],
                ident_t[:],
            )

        # ---- b = ln(cor), w = ln(1-(1-EPS)*ow), a = ln(cor+ow), from PSUM ----
        # ---- cum = cumsum_t(w): single scan along free dim ----
        cum_ct = pool.tile([C, T], FP, tag="cumct")
        nc.vector.tensor_tensor_scan(
            out=cum_ct[:],
            data0=ones_t[:],
            data1=pk_ps[C : 2 * C, :],
            initial=0.0,
            op0=ALU.mult,
            op1=ALU.add,
        )

        b_ct = pool.tile([C, T], FP, tag="b")
        nc.scalar.activation(b_ct[:], pk_ps[0:C, :], AF.Ln)
        a_ct = pool.tile([C, T], FP, tag="a")
        nc.scalar.activation(a_ct[:], pk_ps[2 * C : 3 * C, :], AF.Ln)

        # ---- shift rows first: s1 = u[0, :], s2 = v[0, :] (tiny, unblocks
        # the GpSimd broadcasts while the full u/v subs run) ----
        u0_t = pool.tile([1, T], FR, tag="u0")
        nc.vector.tensor_sub(u0_t[:], a_ct[0:1, :], cum_ct[0:1, :])
        v0_t = pool.tile([1, T], FR, tag="v0")
        nc.vector.tensor_add(v0_t[:], b_ct[0:1, :], cum_ct[0:1, :])
        onesr = pool.tile([1, P], FR, tag="onesr")
        nc.vector.tensor_copy(onesr[:], ones_t[0:1, :P])
        s1b = psum.tile([C, T], FP, tag="s1b")
        nc.tensor.matmul(s1b[:], onesr[:, :C], u0_t[:], start=True, stop=True)
        s2b = psum.tile([C, T], FP, tag="s2b")
        nc.tensor.matmul(s2b[:], onesr[:, :C], v0_t[:], start=True, stop=True)

        # ---- s1col[p, i] = s1[128i + p] via DRAM roundtrip (scalar queue) ----
        nc.scalar.dma_start(m1s, u0_t[:])
        s1col = pool.tile([P, NB], FR, tag="s1col")
        nc.scalar.dma_start(s1col[:], m1s.rearrange("o (i p) -> (o p) i", p=P))

        # ---- u = a - cum, v = b + cum ----
        u_ct = pool.tile([C, T], FP, tag="u")
        nc.vector.tensor_sub(u_ct[:], a_ct[:], cum_ct[:])
        v_ct = pool.tile([C, T], FP, tag="v")
        nc.vector.tensor_add(v_ct[:], b_ct[:], cum_ct[:])

        s2bc = psum.tile([P, T], FP, tag="s2bc")
        nc.tensor.matmul(s2bc[:], onesr[:], v0_t[:], start=True, stop=True)

        # ---- uh = exp(u - s1), vh = exp(v - s2), rounded to f32r ----
        u2_ct = pool.tile([C, T], FP, tag="u2")
        nc.vector.tensor_sub(u2_ct[:], u_ct[:], s1b[:])
        v2_ct = pool.tile([C, T], FP, tag="v2")
        nc.vector.tensor_sub(v2_ct[:], v_ct[:], s2b[:])
        uh_ct = pool.tile([C, T], FR, tag="uh")
        nc.scalar.activation(uh_ct[:], u2_ct[:], AF.Exp)
        vh_ct = pool.tile([C, T], FR, tag="vh")
        nc.scalar.activation(vh_ct[:], v2_ct[:], AF.Exp)

        # ---- per t1-block: S = uh_i^T @ vh ; out = ln S + s1 + s2 ----
        for i in range(NB):
            lo = P * i
            s_ps = psum_s.tile([P, T], FP, tag="s")
            nc.tensor.matmul(
                s_ps[:, lo:],
                uh_ct[:, lo : lo + P],
                vh_ct[:, lo:],
                start=True,
                stop=True,
            )
            lns_t = pp.tile([P, T], FP, tag="lns")
            nc.scalar.activation(lns_t[:, lo:], s_ps[:, lo:], AF.Ln)
            o_t = pp.tile([P, T], FP, tag="o")
            nc.vector.scalar_tensor_tensor(
                out=o_t[:, lo:],
                in0=lns_t[:, lo:],
                scalar=s1col[:, i : i + 1],
                in1=s2bc[:, lo:],
                op0=ALU.add,
                op1=ALU.add,
            )
            me = nc.gpsimd if i < 2 else nc.vector
            me.tensor_mul(o_t[:, lo : lo + P], o_t[:, lo : lo + P], mask_t[:])
            dmae = nc.sync if i % 2 == 0 else nc.scalar
            dmae.dma_start(out[lo : lo + P, lo:], o_t[:, lo:])
            if WRITE_ZEROS and i > 0:
                dmae.dma_start(out[lo : lo + P, :lo], zero_t[:, :lo])


def _consts():
    ident = np.eye(P, dtype=np.float32)
    # mask[p, q] = 1 where q > p (strict upper triangle of the diagonal block)
    maskt = np.triu(np.ones((P, P), dtype=np.float32), k=1)
    oness = np.ones((C, T), dtype=np.float32)
    zeros = np.zeros((P, T - P), dtype=np.float32)
    return {"ident": ident, "maskt": maskt, "oness": oness, "zeros": zeros}


def kernel(coref: np.ndarray, overwrite: np.ndarray) -> np.ndarray:
    B = coref.shape[0]
    assert coref.shape == (B, T, C) and overwrite.shape == (B, T, C)
    if "nc" not in _CACHE:
        _CACHE["nc"] = _build()
    nc = _CACHE["nc"]
    consts = _consts()
    in_maps = [
        {
            "corow": np.concatenate(
                [
                    np.ascontiguousarray(coref[b], dtype=np.float32),
                    np.ascontiguousarray(overwrite[b], dtype=np.float32),
                ],
                axis=1,
            ),
            **consts,
        }
        for b in range(B)
    ]
    res = run_bass_kernel_spmd(nc, in_maps, core_ids=list(range(B)))
    return np.stack([r["out"] for r in res.results], axis=0)
